# revision 39
# baseline (speedup 1.0000x reference)
"""Trainium2 Bass kernel for nn_Attention (pooling attention head).

Reference computation (per batch b):
    score[t]  = hidden[t,:] @ W_score @ hidden[-1,:]        # via u = W_score @ h_t
    attn      = softmax(score)
    context   = sum_t attn[t] * hidden[t,:]
    out       = tanh(concat(context, h_t) @ W_out)

Key optimization: reassociate (hidden @ W_score) @ h_t into
hidden @ (W_score @ h_t) so the kernel is a single memory-bound streaming
pass over hidden_states (32 MB fp32 per core -> ~90us HBM floor).

Sharding: data-parallel over batch, 8 batches per NeuronCore, no
collectives. Each core returns its [8, 128] slice of the output.

Layout: partition p holds t-rows p*16 .. p*16+15; column j of S/P maps to
t = p*16 + j. Softmax is order-agnostic and the context contraction sums
over all (p, j), so the remapping is transparent.

Pipeline structure (software-pipelined): iteration b issues batch b's
score pass (DVE scalar_tensor_tensor, fused mul+accum) BEFORE batch
b-1's softmax stats / context, so the cross-engine stat chain hides
inside the next score window. u is broadcast on-chip (PE transpose +
rank-1 ones matmul), so hidden prefetch never stalls on a DRAM staging
round trip. 1/L is folded into the PSUM->SBUF context-row copy (ACT
per-partition scale), keeping the per-batch DVE budget (~10.6us) under
the per-batch DMA cadence (~11.2us).
"""

import os

os.environ.setdefault("MYCRO_LOCAL_CACHE", "1")

from contextlib import ExitStack

import numpy as np

import concourse.bass as bass
import concourse.tile as tile
from concourse import bacc, mybir
from concourse.bass_utils import run_bass_kernel_spmd
from concourse.masks import make_identity
from concourse.tile_rust import add_dep_helper

B, T, H, UNITS = 64, 2048, 512, 128
NCORES = 8
BL = B // NCORES  # local batches per core
NT = T // 128  # 16 t-tiles per batch

F32 = mybir.dt.float32
BF16 = mybir.dt.bfloat16


def _kernel_body(tc: tile.TileContext, out, hs, ws, wo):
    nc = tc.nc
    with ExitStack() as ctx:
        singles = ctx.enter_context(tc.tile_pool(name="singles", bufs=1))
        hid_pool = ctx.enter_context(tc.tile_pool(name="hid", bufs=14))
        work = ctx.enter_context(tc.tile_pool(name="work", bufs=6))
        small = ctx.enter_context(tc.tile_pool(name="small", bufs=3))
        ps_setup = ctx.enter_context(
            tc.tile_pool(name="ps_setup", bufs=1, space="PSUM")
        )
        ps_ctx = ctx.enter_context(tc.tile_pool(name="ps_ctx", bufs=2, space="PSUM"))
        ps_stat = ctx.enter_context(tc.tile_pool(name="ps_stat", bufs=2, space="PSUM"))
        ps_ubc = ctx.enter_context(tc.tile_pool(name="ps_ubc", bufs=2, space="PSUM"))

        ident = singles.tile([128, 128], F32)
        make_identity(nc, ident)

        # ---- load weights / last-timestep rows --------------------------
        ht_sb = singles.tile([BL, H], F32)  # h_t = hidden[:, -1, :]
        nc.sync.dma_start(out=ht_sb, in_=hs[:, T - 1, :])
        # W_score in 4 row-chunk DMAs so the transpose + u-matmul pipeline
        # can start on chunk r as soon as it lands
        ws_sb = singles.tile([128, 4, H], F32)  # W_score rows r*128+p
        ws_v = ws.rearrange("(r p) k -> p r k", p=128)
        ws_dmas = [
            nc.sync.dma_start(out=ws_sb[:, r : r + 1, :], in_=ws_v[:, r : r + 1, :])
            for r in range(4)
        ]
        ws_dma = ws_dmas[-1]

        # ---- h_t^T: htT_sb[p, c, b] = h_t[b, c*128+p]
        htT_sb = singles.tile([128, 4, BL], F32)
        for c in range(4):
            pst = ps_stat.tile([128, BL], F32, tag="stat")
            nc.tensor.transpose(
                pst, ht_sb[:, c * 128 : (c + 1) * 128], ident[:BL, :BL]
            )
            nc.scalar.copy(htT_sb[:, c, :], pst)

        # ---- W_score^T + u, pipelined per W_score row-chunk -------------
        # wsT_sb[p, kc, h] = W_score[h, kc*128+p]; chunk r's transposes
        # feed u_sb[p, r, b] = u[b][r*128+p] immediately
        wsT_sb = singles.tile([128, 4, H], F32)
        u_sb = singles.tile([128, 4, BL], F32)
        for r in range(4):
            for c in range(4):
                pst = ps_stat.tile([128, 128], F32, tag="stat")
                nc.tensor.transpose(pst, ws_sb[:, r, c * 128 : (c + 1) * 128], ident)
                # alternate copy engines to halve the setup chain latency
                if c % 2 == 0:
                    nc.scalar.copy(wsT_sb[:, c, r * 128 : (r + 1) * 128], pst)
                else:
                    nc.vector.tensor_copy(
                        out=wsT_sb[:, c, r * 128 : (r + 1) * 128], in_=pst
                    )
            psu = ps_stat.tile([128, BL], F32, tag="stat")
            for kc in range(4):
                nc.tensor.matmul(
                    psu,
                    lhsT=wsT_sb[:, kc, r * 128 : (r + 1) * 128],
                    rhs=htT_sb[:, kc, :],
                    start=(kc == 0),
                    stop=(kc == 3),
                )
            nc.scalar.copy(u_sb[:, r, :], psu)

        # W_out is only needed at the very end; load it off the critical path
        wout_sb = singles.tile([128, 8, UNITS], F32)  # W_out rows c*128+p
        nc.sync.dma_start(out=wout_sb, in_=wo.rearrange("(c p) j -> p c j", p=128))

        # preT_sb[p, c, b]: transposed concat(context, h_t); ht half now
        preT_sb = singles.tile([128, 8, BL], F32)
        for c in range(4):
            nc.vector.tensor_copy(out=preT_sb[:, 4 + c, :], in_=htT_sb[:, c, :])

        # ones rows for PE-based partition broadcasts
        ones_sb = singles.tile([1, 128], F32)
        nc.vector.memset(ones_sb, 1.0)
        ones_bf = singles.tile([1, 128], BF16)
        nc.vector.memset(ones_bf, 1.0)

        # ---- hoisted u broadcasts: u_bcs[b][p, h] = u[b][h] --------------
        # built during the DMA lead-in while PE is otherwise idle, so the
        # per-batch loop has no score->broadcast->context coupling
        u_bcs = singles.tile([128, BL, H], BF16)
        for b in range(BL):
            u_row = small.tile([1, H], BF16, tag="urow")
            for hc in range(4):
                tpu = ps_stat.tile([1, 128], F32, tag="stat")
                nc.tensor.transpose(tpu, u_sb[:, hc, b : b + 1], ident)
                nc.scalar.copy(u_row[0:1, hc * 128 : (hc + 1) * 128], tpu)
            psb = ps_ubc.tile([128, H], F32, tag="ubc")
            nc.tensor.matmul(psb, lhsT=ones_bf, rhs=u_row, start=True, stop=True)
            nc.scalar.copy(u_bcs[:, b, :], psb)

        # ---- software-pipelined per-batch loop --------------------------
        # iteration b: issue DMA+u_bc+score for batch b, then finish batch
        # b-1 (softmax stats, exp, context, preT insertion).
        prev = None
        for b in range(BL + 1):
            if b < BL:
                # whole-batch load with inline fp32->bf16 cast (SWDGE),
                # split in two halves for finer pipelining
                hs_v = hs[b].rearrange("(p n) h -> p n h", p=128)
                hid_halves = []
                for half in range(2):
                    hid_bf = hid_pool.tile([128, NT // 2, H], BF16, tag="hid")
                    di = nc.gpsimd.dma_start(
                        out=hid_bf, in_=hs_v[:, half * 8 : half * 8 + 8, :]
                    )
                    if b == 0 and half == 0:
                        # DMA has ~60us of slack; giving W_score a clean head
                        # start shortens the u-chain (and thus score(0)) by
                        # more than the hid stream loses
                        add_dep_helper(
                            di.ins,
                            ws_dma.ins,
                            sync=True,
                            reason="let W_score land before hid flood",
                        )
                    hid_halves.append(hid_bf)

                # score columns, first half. Alternate between the fused
                # DVE op (1x rate, self-contained) and a plain DVE mul
                # (2x rate) whose reduction runs on ACT via copy-accum:
                # DVE ~8.9us/batch and ACT ~8us/batch, both under the
                # ~11us DMA cadence.
                S = small.tile([128, NT], F32, tag="S")
                dump = work.tile([128, H], BF16, tag="dump")
                for j in range(NT // 2):
                    src = hid_halves[0][:, j, :]
                    if j % 2 == 0:
                        prod = work.tile([128, H], BF16, tag="prod_s")
                        nc.vector.scalar_tensor_tensor(
                            prod,
                            src,
                            1.0,
                            u_bcs[:, b, :],
                            op0=mybir.AluOpType.mult,
                            op1=mybir.AluOpType.mult,
                            accum_out=S[:, j : j + 1],
                        )
                    else:
                        prod = work.tile([128, H], BF16, tag="prod_a")
                        nc.vector.tensor_mul(prod, src, u_bcs[:, b, :])
                        nc.scalar.activation(
                            dump,
                            prod,
                            mybir.ActivationFunctionType.Copy,
                            accum_out=S[:, j : j + 1],
                        )
                cur = (S, hid_halves)

            if b > 0:
                # batch b-1 softmax stats (max half): the DVE bits sit
                # between the two score halves of batch b, so the exp and
                # the PE/ACT hops overlap the second score half below
                pS, phid = prev
                m_row = small.tile([128, 1], F32, tag="m_row")
                nc.vector.reduce_max(m_row, pS, axis=mybir.AxisListType.X)
                mT_ps = ps_stat.tile([1, 128], F32, tag="stat")
                nc.tensor.transpose(mT_ps, m_row, ident)
                M_sb = small.tile([1, 1], F32, tag="M_sb")
                nc.vector.reduce_max(
                    M_sb, mT_ps[0:1, :], axis=mybir.AxisListType.X
                )
                Mb_ps = ps_stat.tile([128, 1], F32, tag="stat")
                nc.tensor.matmul(
                    Mb_ps, lhsT=ones_sb, rhs=M_sb, start=True, stop=True
                )
                nm = small.tile([128, 1], F32, tag="nm")
                nc.vector.tensor_scalar_mul(nm, Mb_ps, -1.0)

                P = small.tile([128, NT], BF16, tag="P")
                l_row = small.tile([128, 1], F32, tag="l_row")
                nc.scalar.activation(
                    P,
                    pS,
                    mybir.ActivationFunctionType.Exp,
                    bias=nm,
                    scale=1.0,
                    accum_out=l_row,
                )
                lT_ps = ps_stat.tile([1, 128], F32, tag="stat")
                nc.tensor.transpose(lT_ps, l_row, ident)

            if b < BL:
                # score columns, second half (same hybrid split)
                dump2 = work.tile([128, H], BF16, tag="dump")
                for j in range(NT // 2, NT):
                    src = hid_halves[1][:, j - NT // 2, :]
                    # j=15 stays on DVE so ACT carries 7 tiles (~983ns each
                    # incl. the separate accumulator read) vs DVE's 9 ops
                    if j % 2 == 0 or j == NT - 1:
                        prod = work.tile([128, H], BF16, tag="prod_s")
                        nc.vector.scalar_tensor_tensor(
                            prod,
                            src,
                            1.0,
                            u_bcs[:, b, :],
                            op0=mybir.AluOpType.mult,
                            op1=mybir.AluOpType.mult,
                            accum_out=S[:, j : j + 1],
                        )
                    else:
                        prod = work.tile([128, H], BF16, tag="prod_a")
                        nc.vector.tensor_mul(prod, src, u_bcs[:, b, :])
                        nc.scalar.activation(
                            dump2,
                            prod,
                            mybir.ActivationFunctionType.Copy,
                            accum_out=S[:, j : j + 1],
                        )

            if b > 0:
                S, hid_halves = pS, phid
                bb = b - 1
                L_sb = small.tile([1, 1], F32, tag="L_sb")
                nc.vector.reduce_sum(
                    L_sb, lT_ps[0:1, :], axis=mybir.AxisListType.X
                )
                Linv_sb = small.tile([1, 1], F32, tag="Linv_sb")
                nc.vector.reciprocal(Linv_sb, L_sb)

                # unnormalized context row via PE: attention column
                # stationary ([128,1] ldweights ~1 cycle), hid tile moving
                ps_row = ps_ctx.tile([1, H], F32, tag="ctx")
                for j in range(NT):
                    nc.tensor.matmul(
                        ps_row,
                        lhsT=P[:, j : j + 1],
                        rhs=hid_halves[j // 8][:, j % 8, :],
                        start=(j == 0),
                        stop=(j == NT - 1),
                    )
                # normalize by 1/L during the PSUM->SBUF copy (ACT scale)
                sb_row = small.tile([1, H], F32, tag="sbrow")
                nc.scalar.activation(
                    sb_row,
                    ps_row,
                    mybir.ActivationFunctionType.Copy,
                    scale=Linv_sb,
                )
                # row -> preT columns: PE-transpose each 128-chunk
                tp4 = ps_stat.tile([128, 4], F32, tag="stat")
                for c in range(4):
                    nc.tensor.transpose(
                        tp4[:, c : c + 1],
                        sb_row[0:1, c * 128 : (c + 1) * 128],
                        ident[0:1, 0:1],
                    )
                # on ACT so it never blocks the next score pass in DVE's FIFO
                nc.scalar.copy(
                    preT_sb[:, 0:4, bb : bb + 1].rearrange("p c o -> p (c o)"),
                    tp4,
                )

            if b < BL:
                prev = cur

        # ---- final: out = tanh(pre @ W_out) -----------------------------
        psum_out = ps_setup.tile([BL, UNITS], F32, tag="setup")
        for c in range(8):
            nc.tensor.matmul(
                psum_out,
                lhsT=preT_sb[:, c, :],
                rhs=wout_sb[:, c, :],
                start=(c == 0),
                stop=(c == 7),
            )
        y_sb = small.tile([BL, UNITS], F32, tag="y")
        nc.scalar.activation(y_sb, psum_out, mybir.ActivationFunctionType.Tanh)
        nc.sync.dma_start(out=out, in_=y_sb)


def build_nc():
    nc = bacc.Bacc(
        "TRN2",
        target_bir_lowering=False,
        debug=False,
        enable_asserts=False,
        num_devices=NCORES,
    )
    hs = nc.dram_tensor(
        "hidden_states", [BL, T, H], F32, kind="ExternalInput"
    ).ap()
    ws = nc.dram_tensor("W_score", [H, H], F32, kind="ExternalInput").ap()
    wo = nc.dram_tensor("W_out", [2 * H, UNITS], F32, kind="ExternalInput").ap()
    out = nc.dram_tensor("out", [BL, UNITS], F32, kind="ExternalOutput").ap()

    with tile.TileContext(nc) as tc:
        _kernel_body(tc, out, hs, ws, wo)
    nc.compile()
    return nc


_NC = None


def _get_nc():
    global _NC
    if _NC is None:
        _NC = build_nc()
    return _NC


def make_in_maps(hidden_states, W_score, W_out):
    hidden_states = np.ascontiguousarray(
        np.asarray(hidden_states, dtype=np.float32)
    )
    W_score = np.ascontiguousarray(np.asarray(W_score, dtype=np.float32))
    W_out = np.ascontiguousarray(np.asarray(W_out, dtype=np.float32))
    return [
        {
            "hidden_states": hidden_states[i * BL : (i + 1) * BL],
            "W_score": W_score,
            "W_out": W_out,
        }
        for i in range(NCORES)
    ]


def kernel(hidden_states, W_score, W_out):
    nc = _get_nc()
    in_maps = make_in_maps(hidden_states, W_score, W_out)
    res = run_bass_kernel_spmd(nc, in_maps, core_ids=list(range(NCORES)))
    return np.concatenate([res.results[i]["out"] for i in range(NCORES)], axis=0)
